# revision 17
# baseline (speedup 1.0000x reference)
"""DeepseekV2 MoE layer (M=1024, H=1024, N=1024, E=16, top-6 of 8 groups x2)
on 8 Trainium2 NeuronCores.

Sharding: expert parallelism with group-aligned placement. E=16 experts in 8
groups of 2; grouped_topk keeps the top-3 groups and top_k=6 = 3*2 takes ALL
experts of those groups. Core c owns group c (experts 2c, 2c+1): the host
routes (tiny softmax over 16 logits), shards the token set per core (the
"dispatch"), and each core runs both expert MLPs on its tokens. The
shared-expert MLP is tensor-parallel over its intermediate dim (256 of 2048
per core). The host sums the per-core partials (the "combine" step).

Precision: the routed-expert GEMMs run in fp8 e4m3 with DoubleRow perf mode
(K=256 per matmul, ~1.8x PE throughput; global pow2 scales, fp32 PSUM,
descale folded into the silu scale / router-weight / output-cast immediates).
The shared-expert MLP stays fp16: its output enters at weight 1.0 while each
routed expert is attenuated by its softmax score, so fp8 there would
dominate the error budget (simulated 6.5e-2 vs the 1.5e-2 achieved; harness
gate is 2e-2). fp32 PSUM accumulation throughout.

Schedule (PE-roofline-driven, from trace analysis):
- few LARGE DMAs (one DMA's packets fan out across all 16 HW DGE queues, and
  each DMA_DIRECT2D issue costs ~600ns of Sync-queue time regardless of size)
- a dozen dummy matmuls on a scratch tile warm the PE HAM clock-gate
  (1.2->2.4 GHz) while the first real loads are still in flight
- GEMM1 streams token pairs against stationary w1 k-pair chunks
- GEMM2 keeps w2 [n,o]-chunks stationary and streams gtw, so the out
  partition dim is a full 128 (o) instead of W%128 token chunks
- phase order GEMM1 -> shared-s1 -> {shared-s2 interleaved with GEMM2}: the
  slow serialized PSUM drains of s2 ([128,1024] copies) hide under GEMM2's
  PE work, and the kernel tail after the last matmul is one [128,W] copy +
  store; s2 drains alternate Scalar/Vector; stores issue from the Sync queue
"""
import sys

sys.path.insert(0, "/opt/trn_rl_repo")

import numpy as np
import ml_dtypes

import concourse.mybir as mybir
import concourse.tile as tile
from concourse import bacc
from concourse.bass_utils import run_bass_kernel_spmd

P = 128
M = 1024          # tokens
H = 1024          # hidden
NI = 1024         # moe_intermediate
E = 16
N_GROUP = 8
TOPK_GROUP = 3
I_SH = 2048       # shared-expert intermediate (n_shared * moe_intermediate)
ISH_C = I_SH // 8  # per-core shared slice = 256
S_G = 16.0        # fixed fp8 scale for the gtw activations

F32 = mybir.dt.float32
F16 = mybir.dt.float16
F8 = mybir.dt.float8e4
E4NP = ml_dtypes.float8_e4m3  # TRN variant: max normal +-240
DR = mybir.MatmulPerfMode.DoubleRow
AF = mybir.ActivationFunctionType
MULT = mybir.AluOpType.mult

_PROGRAM_CACHE = {}


def _build_program(W, d1, dy):
    """SPMD program for one core; W = token capacity (mult of 16, <=512).
    d1 = 1/(s_x*s_w1) GEMM1 descale; dy = 1/(S_G*s_w2) GEMM2 descale."""
    assert 0 < W <= 512 and W % 16 == 0

    nc = bacc.Bacc("TRN2", target_bir_lowering=False, debug=False, num_devices=8)

    # --- per-core DRAM I/O ---
    # w1c[e*8+j] = [128 h-in-chunk, 16 chunk-slices x 128]: slices 0-7 gate
    # f-chunk j at h-chunk k, slices 8-15 the up projection.
    w1c = nc.dram_tensor("w1c", [16, P, 2048], F8, kind="ExternalInput").ap()
    # w2c[e] = [128 n-in-chunk, n-chunk-major o cols]: block n at cols n*1024
    w2c = nc.dram_tensor("w2c", [2, P, 8 * 1024], F8, kind="ExternalInput").ap()
    xt = nc.dram_tensor("xt", [P, 8, W], F8, kind="ExternalInput").ap()
    ht = nc.dram_tensor("ht", [P, 8 * M], F16, kind="ExternalInput").ap()
    gus = nc.dram_tensor("gus", [P, 8 * 2 * ISH_C], F16, kind="ExternalInput").ap()
    dst = nc.dram_tensor("dst", [P, 2 * H], F16, kind="ExternalInput").ap()
    wab = nc.dram_tensor("wab", [P, 2, W], F32, kind="ExternalInput").ap()
    # y[oc] = [128 o rows of chunk oc, W token cols]  (routed out, transposed)
    y = nc.dram_tensor("y", [8, P, W], F16, kind="ExternalOutput").ap()
    shared = nc.dram_tensor("shared", [M, H], F16, kind="ExternalOutput").ap()

    with tile.TileContext(nc) as tc:
        with (
            tc.tile_pool(name="persist", bufs=1) as persist,
            tc.tile_pool(name="stream", bufs=8) as stream,
            tc.tile_pool(name="work", bufs=2) as work,
            tc.tile_pool(name="drain", bufs=4) as drain,
            tc.tile_pool(name="psum", bufs=2, space="PSUM") as psum,
            tc.tile_pool(name="psumw", bufs=4, space="PSUM") as psumw,
        ):
            # --- PE warm-up: HAM un-throttles after ~3.4us of sustained MMs;
            # run dummies on a scratch tile while the first loads are in
            # flight (results never read).
            zt = persist.tile([P, 640], F16, tag="zt")
            nc.gpsimd.memset(zt[:, :1], 0.0)  # tiny: just creates the tile dep
            with nc.named_scope("warm"):
                wps = psum.tile([P, 1024], F32, space="PSUM", tag="acc")
                for _ in range(8):
                    nc.tensor.matmul(
                        wps[:, :512], zt[:, :P], zt[:, P:640], start=True, stop=True
                    )

            # --- loads: few large DMAs, first-needed first ---
            t_xta = persist.tile([P, 4, W], F8, tag="xta")
            t_xtb = persist.tile([P, 4, W], F8, tag="xtb")
            t_wab = persist.tile([P, 2, W], F32, tag="wab")
            t_w1 = [
                stream.tile([P, 16, P], F8, tag="w1", name=f"w1_{i}")
                for i in range(16)
            ]
            nc.sync.dma_start(out=t_xta[:], in_=xt[:, :4, :])
            nc.sync.dma_start(out=t_w1[0][:], in_=w1c[0])
            nc.sync.dma_start(out=t_xtb[:], in_=xt[:, 4:, :])
            nc.sync.dma_start(out=t_w1[1][:], in_=w1c[1])
            nc.sync.dma_start(out=t_w1[2][:], in_=w1c[2])
            nc.sync.dma_start(out=t_wab[:], in_=wab)
            for i in range(3, 16):
                nc.sync.dma_start(out=t_w1[i][:], in_=w1c[i])
            t_gus = persist.tile([P, 8 * 2 * ISH_C], F16, tag="gus")
            nc.sync.dma_start(out=t_gus[:], in_=gus)
            t_hta = persist.tile([P, 4 * M], F16, tag="hta")
            t_htb = persist.tile([P, 4 * M], F16, tag="htb")
            nc.sync.dma_start(out=t_hta[:], in_=ht[:, :4 * M])
            nc.sync.dma_start(out=t_htb[:], in_=ht[:, 4 * M:])
            t_dst = persist.tile([P, 2 * H], F16, tag="dst")
            nc.sync.dma_start(out=t_dst[:], in_=dst)
            t_w2 = persist.tile([P, 16, 1024], F8, tag="w2")
            for e in range(2):
                nc.sync.dma_start(out=t_w2[:, e * 8:(e + 1) * 8, :], in_=w2c[e])

            def xt_pair(p):  # k-chunk pair (2p, 2p+1) -> [128, 2, W]
                if p < 2:
                    return t_xta[:, 2 * p:2 * p + 2, :]
                return t_xtb[:, 2 * p - 4:2 * p - 2, :]

            # --- GEMM1 (fp8 DoubleRow) + silu*up*weight -> gtw (fp8) ---
            t_gtw = persist.tile([P, 16, W], F8, tag="gtw")
            with nc.named_scope("gemm1"):
                for e in range(2):
                    for j in range(8):
                        w1t = t_w1[e * 8 + j]
                        pg = psumw.tile([P, W], F32, space="PSUM", tag="accw")
                        pu = psumw.tile([P, W], F32, space="PSUM", tag="accw")
                        for p in range(4):
                            nc.tensor.matmul(
                                pg[:], w1t[:, 2 * p:2 * p + 2, :], xt_pair(p),
                                start=(p == 0), stop=(p == 3), perf_mode=DR,
                            )
                        for p in range(4):
                            nc.tensor.matmul(
                                pu[:], w1t[:, 8 + 2 * p:8 + 2 * p + 2, :],
                                xt_pair(p),
                                start=(p == 0), stop=(p == 3), perf_mode=DR,
                            )
                        sg = work.tile([P, W], F32, tag="sg")
                        nc.scalar.activation(
                            out=sg[:], in_=pg[:], func=AF.Silu, scale=float(d1)
                        )
                        gt = work.tile([P, W], F32, tag="gt")
                        nc.vector.tensor_tensor(
                            out=gt[:], in0=sg[:], in1=pu[:], op=MULT
                        )
                        # wab carries router weight * d1 * S_G; fp8 output
                        nc.vector.tensor_tensor(
                            out=t_gtw[:, e * 8 + j, :],
                            in0=gt[:],
                            in1=t_wab[:, e, :],
                            op=MULT,
                        )

            def ht_slice(k, mh):
                t = t_hta if k < 4 else t_htb
                return t[:, (k % 4) * M + mh * 512:(k % 4) * M + (mh + 1) * 512]

            # --- shared expert s1, fp16 (TP slice over intermediate dim) ---
            t_gts = persist.tile([P, 2 * M], F16, tag="gts")
            with nc.named_scope("shared1"):
                for ip in range(2):  # 128-row i-slices of the 256-wide slice
                    ag = psum.tile([P, 1024], F32, space="PSUM", tag="acc")
                    au = psum.tile([P, 1024], F32, space="PSUM", tag="acc")
                    for mh in range(2):
                        for k in range(8):
                            nc.tensor.matmul(
                                ag[:, mh * 512:(mh + 1) * 512],
                                t_gus[:, k * 512 + ip * P:k * 512 + (ip + 1) * P],
                                ht_slice(k, mh),
                                start=(k == 0),
                                stop=(k == 7),
                            )
                    for mh in range(2):
                        for k in range(8):
                            nc.tensor.matmul(
                                au[:, mh * 512:(mh + 1) * 512],
                                t_gus[:, k * 512 + 256 + ip * P:
                                      k * 512 + 256 + (ip + 1) * P],
                                ht_slice(k, mh),
                                start=(k == 0),
                                stop=(k == 7),
                            )
                    ss = work.tile([P, M], F32, tag="ss")
                    nc.scalar.activation(out=ss[:], in_=ag[:], func=AF.Silu)
                    nc.vector.tensor_tensor(
                        out=t_gts[:, ip * M:(ip + 1) * M],
                        in0=ss[:],
                        in1=au[:],
                        op=MULT,
                    )

            # --- shared s2 (fp16) interleaved with GEMM2 (fp8 DoubleRow).
            # The PE queue is in-order, so GEMM2 o-chunk groups (inputs long
            # since ready) are emitted between s2 m-chunk groups to cover
            # s2's drain latency.
            def s2_group(mc):
                acc = psum.tile(
                    [P, 1024], F32, space="PSUM", tag="acc", name=f"s2_{mc}"
                )
                for oh in range(2):
                    for ip in range(2):
                        nc.tensor.matmul(
                            acc[:, oh * 512:(oh + 1) * 512],
                            t_gts[:, ip * M + mc * P:ip * M + (mc + 1) * P],
                            t_dst[:, ip * H + oh * 512:ip * H + (oh + 1) * 512],
                            start=(ip == 0),
                            stop=(ip == 1),
                        )
                sh = drain.tile([P, H], F16, tag="sh", name=f"sh_{mc}")
                nc.scalar.copy(out=sh[:], in_=acc[:])
                nc.sync.dma_start(out=shared[mc * P:(mc + 1) * P, :], in_=sh[:])

            def gemm2_group(oc):
                occ = psumw.tile(
                    [P, W], F32, space="PSUM", tag="accw", name=f"occ_{oc}"
                )
                for e in range(2):
                    for p in range(4):
                        idx = e * 8 + 2 * p
                        nc.tensor.matmul(
                            occ[:],
                            t_w2[:, idx:idx + 2, oc * P:(oc + 1) * P],
                            t_gtw[:, idx:idx + 2, :],
                            start=(e == 0 and p == 0),
                            stop=(e == 1 and p == 3),
                            perf_mode=DR,
                        )
                yt = drain.tile([P, W], F16, tag="yt", name=f"yt_{oc}")
                nc.vector.tensor_scalar_mul(yt[:], occ[:], float(dy))
                nc.sync.dma_start(out=y[oc], in_=yt[:])

            with nc.named_scope("s2_gemm2"):
                gemm2_group(0)
                for pair in range(4):
                    s2_group(2 * pair)
                    s2_group(2 * pair + 1)
                    gemm2_group(pair + 1)
                for oc in range(5, 8):
                    gemm2_group(oc)

    nc.compile()
    return nc


def _get_program(W, d1, dy):
    key = (W, float(d1), float(dy))
    if key not in _PROGRAM_CACHE:
        _PROGRAM_CACHE[key] = _build_program(W, d1, dy)
    return _PROGRAM_CACHE[key]


def _route(hidden_states, gate_w):
    """Numpy replica of grouped_topk: softmax -> per-group max -> top-3 groups.
    With E=16, n_group=8, topk_group=3, top_k=6, the top-6 experts are exactly
    all experts of the top-3 groups and keep their softmax scores."""
    lg = hidden_states @ gate_w.T
    lg = lg - lg.max(axis=1, keepdims=True)
    sc = np.exp(lg)
    sc /= sc.sum(axis=1, keepdims=True)
    gsc = sc.reshape(M, N_GROUP, E // N_GROUP).max(axis=2)
    top = np.argsort(-gsc, axis=1, kind="stable")[:, :TOPK_GROUP]
    gmask = np.zeros((M, N_GROUP), bool)
    np.put_along_axis(gmask, top, True, axis=1)
    return sc.astype(np.float32), gmask


def _pow2scale(a, target=208.0):
    return float(2.0 ** np.floor(np.log2(target / np.abs(a).max())))


def _q8(a, s):
    return np.clip(a * s, -240.0, 240.0).astype(E4NP)


def _chunk_major(a, nchunk):
    """[nchunk*P, C] -> [P, nchunk*C] with chunk k at cols k*C."""
    c = a.shape[1]
    return np.ascontiguousarray(
        a.reshape(nchunk, P, c).transpose(1, 0, 2).reshape(P, nchunk * c)
    )


def _prep_core(c, hidden, ht_l, w1, w2, sgu_t, sd_t, sc, gmask, W, scales):
    s_x, s_w1, s_w2 = scales
    d1 = 1.0 / (s_x * s_w1)
    tok = np.nonzero(gmask[:, c])[0].astype(np.int32)
    n = len(tok)
    wa = np.zeros(W, np.float32)
    wb = np.zeros(W, np.float32)
    wa[:n] = sc[tok, 2 * c] * (d1 * S_G)
    wb[:n] = sc[tok, 2 * c + 1] * (d1 * S_G)

    xp = np.zeros((W, H), np.float32)
    xp[:n] = hidden[tok]
    xtc = _chunk_major(_q8(xp.T, s_x), 8).reshape(P, 8, W)

    w1c = np.empty((16, P, 2048), E4NP)
    w2c = np.empty((2, P, 8 * 1024), E4NP)
    for i, e in enumerate((2 * c, 2 * c + 1)):
        # block (f_chunk j, h_chunk k): [h_in (part), f_in] = w1[e][j*128+q, k*128+p]
        w1r = (
            _q8(w1[e], s_w1).reshape(16, P, 8, P).transpose(0, 3, 2, 1)
            .reshape(16, P, 8 * P)
        )
        w1c[i * 8:(i + 1) * 8, :, :1024] = w1r[:8]
        w1c[i * 8:(i + 1) * 8, :, 1024:] = w1r[8:]
        w2c[i] = _chunk_major(_q8(np.ascontiguousarray(w2[e].T), s_w2), 8)

    gusc = _chunk_major(
        np.concatenate(
            (
                sgu_t[:, c * ISH_C:(c + 1) * ISH_C],
                sgu_t[:, I_SH + c * ISH_C:I_SH + (c + 1) * ISH_C],
            ),
            axis=1,
        ).astype(np.float16),
        8,
    )  # [P, 8*512]
    dstc = _chunk_major(
        sd_t[c * ISH_C:(c + 1) * ISH_C, :].astype(np.float16), 2
    )  # [P, 2H]

    return tok, {
        "w1c": w1c,
        "w2c": w2c,
        "xt": xtc,
        "ht": ht_l,
        "gus": gusc,
        "dst": dstc,
        "wab": np.ascontiguousarray(
            np.stack((np.broadcast_to(wa, (P, W)), np.broadcast_to(wb, (P, W))),
                     axis=1)
        ),
    }


def _run(inputs, trace=False):
    hidden = np.ascontiguousarray(np.asarray(inputs["hidden_states"], np.float32))
    gate_w = np.asarray(inputs["gate_w"], np.float32)
    w1 = np.asarray(inputs["w1"], np.float32)
    w2 = np.asarray(inputs["w2"], np.float32)
    sgu = np.asarray(inputs["shared_gate_up"], np.float32)
    sd = np.asarray(inputs["shared_down"], np.float32)

    sc, gmask = _route(hidden, gate_w)
    counts = gmask.sum(axis=0)
    W = int(min(512, -(-int(counts.max()) // 16) * 16))
    assert counts.max() <= W, f"capacity overflow: {counts}"

    s_x = _pow2scale(hidden)
    s_w1 = _pow2scale(w1)
    s_w2 = _pow2scale(w2)
    d1 = 1.0 / (s_x * s_w1)
    dy = 1.0 / (S_G * s_w2)

    ht_l = _chunk_major(hidden.T.astype(np.float16), 8)  # [P, 8M]
    sgu_t = np.ascontiguousarray(sgu.T)  # [H, 2*I_SH]
    sd_t = np.ascontiguousarray(sd.T)    # [I_SH, H]

    nc = _get_program(W, d1, dy)
    toks = []
    in_maps = []
    for c in range(8):
        tok, im = _prep_core(
            c, hidden, ht_l, w1, w2, sgu_t, sd_t, sc, gmask, W,
            (s_x, s_w1, s_w2),
        )
        toks.append(tok)
        in_maps.append(im)
    res = run_bass_kernel_spmd(nc, in_maps, core_ids=list(range(8)), trace=trace)

    out = np.zeros((M, H), np.float32)
    for c in range(8):
        out += res.results[c]["shared"].astype(np.float32)
        tok = toks[c]
        yt = res.results[c]["y"].reshape(8 * P, W)  # [H, W] (transposed)
        out[tok] += yt[:, :len(tok)].T.astype(np.float32)
    return out, res


def kernel(**inputs):
    out, _ = _run(inputs, trace=False)
    return out
